# revision 32
# baseline (speedup 1.0000x reference)
"""Trainium2 Bass kernel for nn_BilinearPolicy (dense_mlp).

Math (reference):
  ob = trunk_obs(obs)      : [B,256] -> 2048 -> 2048 -> 2048 -> 16384 (ReLU between)
  dl = trunk_dlt(deltas)   : same shapes, different weights
  pred[b,a] = sum_f ob[b, a*512+f] * dl[b, f*32+a]            : [B, 32]

Strategy:
  * Data-parallel over batch: 8 cores x 512 rows, zero collectives.
  * Feature-major activations on chip ([feat(part), batch(free)]), so the
    torch-layout weights [din, dout] are used directly as matmul lhsT tiles
    and no transposes are ever needed. Inputs are transposed on host.
  * bf16 matmuls with fp32 PSUM accumulation. Biases applied during the
    mandatory PSUM->SBUF eviction: trunk o on the Scalar engine, trunk d on
    the DVE (tensor_scalar), so neither queue's serial backlog stalls the PE
    at group boundaries.
  * dl's last-layer weight columns are permuted on host from (f,a) to (a,f)
    ordering, so the bilinear diagonal becomes: elementwise multiply of the
    two [16384, 512] outputs, then a segmented 512-row partition reduction.
    The 4 z-tiles of an action are accumulated on the DVE; one one-hot mask
    matmul per action accumulates pred^T [32, 512] in a single PSUM tile.
  * The PE is the bottleneck (~98% of the bf16 roofline); the remaining
    slack is schedule edges, attacked here:
      - inputs + L0 weights stream in first-use order across both HWDGE
        queues (they share DMA bandwidth, so order trumps placement), with
        a small first obs chunk so the first matmul starts ~11us in;
      - all 32 mask matmuls are deferred to the very end, where they run
        back-to-back at ~26ns each (one pipeline break total) overlapping
        the final DVE chain;
      - the last action's final m-tile runs as two 256-column PSUM groups
        so half 0's evict/mult/transpose/reduce chain hides under half 1's
        matmuls, leaving a ~1.5us end-of-kernel tail;
      - pred columns for actions 0..30 are copied/DMA'd while that chain
        drains; only 16 columns ride the final dependency tail.
"""

from contextlib import ExitStack

import numpy as np
import ml_dtypes

B, OBS, H, F, A = 4096, 256, 2048, 512, 32
DOUT = F * A            # 16384
NCORES = 8
BPC = B // NCORES       # 512 batch rows per core
P = 128

KT = [2, 16, 16, 16]    # k-tiles per layer
MT = [16, 16, 16, 128]  # m-tiles per layer
GR = [8, 4, 4, 4]       # m-tiles grouped per weight DMA
GR0 = {"o": 2, "d": 8}  # L0 chunks per trunk; a small first obs chunk lets
                        # the first matmul start ~1.5us earlier

BF16 = ml_dtypes.bfloat16

# Filled with the BassKernelResults of the most recent run (for test harness).
LAST_RESULTS = None


def _tile_weight(w, G):
    """[D1, D2] fp32 -> [D2/(128G), 128(k), G*D1] bf16. Slice
    [:, (g*Kt + j)*128 : +128] of group tile mtg is the lhsT for
    k-tile j of m-tile mtg*G+g; every partition line is contiguous."""
    d1, d2 = w.shape
    kt, mt = d1 // P, d2 // P
    wt = w.reshape(kt, P, mt, P).transpose(2, 1, 0, 3)      # [mt, k, j, m]
    wt = wt.reshape(mt // G, G, P, kt * P).transpose(0, 2, 1, 3)
    wt = wt.reshape(mt // G, P, G * kt * P)
    return np.ascontiguousarray(wt.astype(BF16))


def _tile_bias(b):
    """[D2] fp32 -> [128, D2/128] fp32; column mt holds bias for m-tile mt
    as a per-partition scalar."""
    return np.ascontiguousarray(b.reshape(-1, P).T.astype(np.float32))


def _build_program():
    import concourse.bass as bass
    import concourse.tile as tile
    from concourse import bacc, mybir
    from concourse.alu_op_type import AluOpType
    from concourse.bass import ts

    dt = mybir.dt
    AF = mybir.ActivationFunctionType

    nc = bacc.Bacc(
        "TRN2",
        target_bir_lowering=False,
        debug=False,
        enable_asserts=True,
        num_devices=NCORES,
    )

    def din(name, shape, dtype):
        return nc.dram_tensor(name, shape, dtype, kind="ExternalInput").ap()

    x_d = {
        "o": din("xo", [P, 2 * BPC], dt.bfloat16),
        "d": din("xd", [P, 2 * BPC], dt.bfloat16),
    }
    w_d = {}
    b_d = {}
    for t in ("o", "d"):
        for l in range(4):
            g = GR0[t] if l == 0 else GR[l]
            w_d[t, l] = din(f"{t}w{l}", [MT[l] // g, P, g * KT[l] * P],
                            dt.bfloat16)
            b_d[t, l] = din(f"{t}b{l}", [P, MT[l]], dt.float32)
    masks_d = din("masks", [P, A], dt.bfloat16)
    pred_d = nc.dram_tensor("pred", [A, BPC], dt.float32, kind="ExternalOutput").ap()

    with tile.TileContext(nc) as tc, ExitStack() as ctx:
        const = ctx.enter_context(tc.tile_pool(name="const", bufs=1))
        wp = ctx.enter_context(tc.tile_pool(name="wp", bufs=4))
        act = ctx.enter_context(tc.tile_pool(name="act", bufs=1))
        ev = ctx.enter_context(tc.tile_pool(name="ev", bufs=10))
        ps = ctx.enter_context(tc.tile_pool(name="ps", bufs=7, space="PSUM"))
        psp = ctx.enter_context(tc.tile_pool(name="psp", bufs=1, space="PSUM"))

        # weight DMAs round-robin over both HWDGE queues so supply isn't
        # capped by a single queue's descriptor rate (the queues share the
        # underlying DMA bandwidth, so order matters more than placement)
        dma_engs = [nc.sync, nc.scalar]
        rr = [0]

        def wdma(dst, src):
            dma_engs[rr[0] % len(dma_engs)].dma_start(dst, src)
            rr[0] += 1

        # inputs + L0 weights first, in two chunks split across both HWDGE
        # queues in first-use order so the PE can start within a few us;
        # other small constants go on the gpsimd SWDGE queue
        x_sb = {}
        w0_sb = {}
        for t in ("o", "d"):
            chunk = GR0[t] * KT[0] * P
            x_sb[t] = const.tile([P, 2, BPC], dt.bfloat16,
                                 tag=f"x{t}", name=f"x{t}")
            wdma(x_sb[t][:], x_d[t].rearrange("p (k n) -> p k n", n=BPC))
            w0_sb[t] = const.tile([P, MT[0] * KT[0] * P], dt.bfloat16,
                                  tag=f"w0{t}", name=f"w0{t}")
            for c in range(MT[0] // GR0[t]):
                wdma(w0_sb[t][:, c * chunk:(c + 1) * chunk], w_d[t, 0][c])
        bias_sb = {}
        for t in ("o", "d"):
            for l in range(4):
                bias_sb[t, l] = const.tile([P, MT[l]], dt.float32,
                                           tag=f"b{t}{l}", name=f"b{t}{l}")
                nc.gpsimd.dma_start(bias_sb[t, l][:], b_d[t, l][:])
        masks_sb = const.tile([P, A], dt.bfloat16, tag="masks")
        nc.gpsimd.dma_start(masks_sb[:], masks_d[:])

        def evict(t, dst, pt, l, mt, relu):
            """PSUM->SBUF with bias: trunk o on Scalar, trunk d on DVE."""
            bias = bias_sb[t, l][:, mt:mt + 1]
            if t == "o":
                nc.scalar.activation(dst, pt, AF.Relu if relu else AF.Identity,
                                     bias=bias)
            elif relu:
                nc.vector.tensor_scalar(dst, pt, bias, 0.0,
                                        AluOpType.add, AluOpType.max)
            else:
                nc.vector.tensor_scalar_add(dst, pt, bias)

        # ---- Trunks: layers 0..2 with ReLU, feature-major throughout.
        # The two trunks are interleaved layer-by-layer so the PE has twice
        # the work per phase start, covering the weight-stream warm-up.
        cur = dict(x_sb)
        for l in range(3):
            for t in ("o", "d"):
                out_t = act.tile([P, MT[l], BPC], dt.bfloat16,
                                 tag=f"h{t}{l % 2}", name=f"h{t}{l}")
                for mtg in range(MT[l] // GR[l]):
                    if l == 0:
                        wt = w0_sb[t]
                    else:
                        wt = wp.tile([P, GR[l] * KT[l] * P], dt.bfloat16,
                                     tag="wbig")
                        wdma(wt[:], w_d[t, l][mtg])
                    for g in range(GR[l]):
                        mt = mtg * GR[l] + g
                        wcol = mt if l == 0 else g  # L0 tile is fully resident
                        pt = ps.tile([P, BPC], dt.float32, tag="mm")
                        for j in range(KT[l]):
                            nc.tensor.matmul(
                                pt[:], wt[:, ts(wcol * KT[l] + j, P)],
                                cur[t][:, j, :],
                                start=(j == 0), stop=(j == KT[l] - 1),
                            )
                        evict(t, out_t[:, mt, :], pt[:], l, mt, relu=True)
                cur[t] = out_t
        h = cur

        # ---- Layer 3 + bilinear diagonal, fused per 128-feature tile.
        # The partition reduction of z = ob*dl runs on the DVE: each z tile
        # is 32x32 block-transposed and free-axis reduced to R [128, 16]
        # (R[32i+r, j] = sum_c z[32i+c, 32j+r]), the per-action R tiles are
        # summed, and a single 16-row matmul against a one-hot mask finishes
        # the cross-block sum: pred4[m, a*16+j] = pred[a, 32j+m]. That
        # matmul moves 16 rows instead of the 512 a direct mask reduction
        # costs, and is deferred into the next action's matmul stream so
        # the PE never waits on the DVE chain. All actions share one PSUM
        # bank (disjoint 16-column slices; start zeroes the whole bank at
        # a==0). pred4 is copied/DMA'd out in two halves; the host undoes
        # the (m, a*16+j) layout.
        pred_ps = psp.tile([A, 16 * A], dt.float32, tag="pred")
        pred_sb = ev.tile([A, 16 * A], dt.float32, tag="predsb", bufs=1)
        pending = []

        def emit_mask(pa, pr):
            nc.tensor.matmul(
                pred_ps[:, pa * 16:(pa + 1) * 16], masks_sb[:], pr[:],
                start=(pa == 0), stop=(pa == A - 1),
            )

        for a in range(A):  # one weight DMA per trunk covers the whole action
            r_acc = None
            wt = {}
            for t in ("o", "d"):
                wt[t] = wp.tile([P, GR[3] * KT[3] * P], dt.bfloat16,
                                tag="wbig", name=f"w3{t}")
                wdma(wt[t][:], w_d[t, 3][a])
            for g in range(GR[3]):
                mt = a * 4 + g
                # the very last g runs as two 256-column halves in separate
                # PSUM groups, so half 0's DVE chain hides under half 1's
                # matmuls and the end-of-kernel tail is one half-chain long
                halves = 2 if (a == A - 1 and g == GR[3] - 1) else 1
                hw = BPC // halves
                for hf in range(halves):
                    cols = slice(hf * hw, (hf + 1) * hw)
                    s = {}
                    for t in ("o", "d"):
                        pt = ps.tile([P, hw], dt.float32, tag="mm")
                        for j in range(KT[3]):
                            nc.tensor.matmul(
                                pt[:], wt[t][:, ts(g * KT[3] + j, P)],
                                h[t][:, j, cols],
                                start=(j == 0), stop=(j == KT[3] - 1),
                            )
                        s[t] = ev.tile([P, hw], dt.bfloat16, tag="evict",
                                       name=f"s{t}")
                        evict(t, s[t][:], pt[:], 3, mt, relu=False)
                    zg = ev.tile([P, hw], dt.bfloat16, tag="ztmp", name="zg")
                    nc.vector.tensor_mul(zg[:], s["o"][:], s["d"][:])
                    zt = ev.tile([P, hw], dt.bfloat16, tag="ztr", name="zt")
                    nc.vector.transpose(zt[:], zg[:])
                    rcols = slice(hf * (16 // halves), (hf + 1) * (16 // halves))
                    with nc.allow_low_precision("32-term DVE reduce, fp32 acc"):
                        if g == 0:
                            r_acc = ev.tile([P, 16], dt.bfloat16, tag="racc",
                                            bufs=A + 1)
                            nc.vector.tensor_reduce(
                                r_acc[:], zt.rearrange("p (j c) -> p j c", c=32),
                                mybir.AxisListType.X, AluOpType.add)
                        else:
                            rg = ev.tile([P, 16 // halves], dt.bfloat16,
                                         tag="rg")
                            nc.vector.tensor_reduce(
                                rg[:], zt.rearrange("p (j c) -> p j c", c=32),
                                mybir.AxisListType.X, AluOpType.add)
                            nc.vector.tensor_add(r_acc[:, rcols],
                                                 r_acc[:, rcols], rg[:])
            pending.append((a, r_acc))
        # all 32 mask matmuls run back-to-back here: one pipeline break
        # total, overlapped with the last action's DVE chain. Columns of
        # actions 0..30 are copied/DMA'd while that chain drains; only the
        # last 16 columns ride the final dependency tail.
        for pa, pr in pending[:-1]:
            emit_mask(pa, pr)
        nc.scalar.activation(pred_sb[:, :16 * (A - 1)],
                             pred_ps[:, :16 * (A - 1)], AF.Copy)
        nc.sync.dma_start(pred_d[:, :16 * (A - 1)], pred_sb[:, :16 * (A - 1)])
        emit_mask(*pending[-1])
        nc.scalar.activation(pred_sb[:, 16 * (A - 1):],
                             pred_ps[:, 16 * (A - 1):], AF.Copy)
        nc.sync.dma_start(pred_d[:, 16 * (A - 1):], pred_sb[:, 16 * (A - 1):])

    nc.compile()
    return nc


def _prep_inputs(inputs):
    """Host-side layout/dtype prep shared across cores + per-core slices."""
    shared = {}

    for t, pfx in (("o", "obs"), ("d", "dlt")):
        for l in range(4):
            w = np.asarray(inputs[f"{pfx}_W{l}"], np.float32)
            b = np.asarray(inputs[f"{pfx}_b{l}"], np.float32)
            if t == "d" and l == 3:
                # permute columns (f,a) -> (a,f) to match obs layout
                w = w.reshape(H, F, A).transpose(0, 2, 1).reshape(H, DOUT)
                b = b.reshape(F, A).T.reshape(DOUT)
            shared[f"{t}w{l}"] = _tile_weight(w, GR0[t] if l == 0 else GR[l])
            shared[f"{t}b{l}"] = _tile_bias(b)

    # mask4[32i + r, m] = (r == m): the 16-row matmul sums R over the four
    # 32-partition blocks, landing pred4[m, j] = pred[a, 32j + m]
    masks = np.zeros((P, A), np.float32)
    for q in range(P):
        masks[q, q % A] = 1.0
    shared["masks"] = np.ascontiguousarray(masks.astype(BF16))

    obsT = np.asarray(inputs["obs"], np.float32).T.astype(BF16)    # [256, 4096]
    dltT = np.asarray(inputs["deltas"], np.float32).T.astype(BF16)

    in_maps = []
    for c in range(NCORES):
        sl = slice(c * BPC, (c + 1) * BPC)
        m = dict(shared)
        m["xo"] = np.ascontiguousarray(
            obsT[:, sl].reshape(2, P, BPC).transpose(1, 0, 2).reshape(P, 2 * BPC))
        m["xd"] = np.ascontiguousarray(
            dltT[:, sl].reshape(2, P, BPC).transpose(1, 0, 2).reshape(P, 2 * BPC))
        in_maps.append(m)
    return in_maps


_PROGRAM = None


def kernel(**inputs):
    global _PROGRAM, LAST_RESULTS
    from concourse.bass_utils import run_bass_kernel_spmd

    if _PROGRAM is None:
        _PROGRAM = _build_program()
    in_maps = _prep_inputs(inputs)
    res = run_bass_kernel_spmd(_PROGRAM, in_maps, list(range(NCORES)))
    LAST_RESULTS = res
    out = np.empty((B, A), np.float32)
    for c in range(NCORES):
        # pred4[m, a*16 + j] = pred[a, 32j + m]
        p4 = res.results[c]["pred"]
        predT = p4.reshape(A, A, 16).transpose(1, 2, 0).reshape(A, BPC)
        out[c * BPC:(c + 1) * BPC] = predT.T
    return out
